# revision 24
# baseline (speedup 1.0000x reference)
"""Center-update (scatter-add) kernel for Trainium2, 8 NeuronCores.

Math: given features [B, D], labels [B], centers [N, D]:
    diff        = (ALPHA - 1) * (centers[labels] - features)
    new_centers = centers.at[labels].add(diff)
which reduces per center row n to
    new_centers[n] = centers[n] * (1 - 0.1*count[n]) + 0.1 * featsum[n]
with count = histogram(labels), featsum = segment-sum of features by label.

Sharding: centers are sharded along N across the 8 cores (12500 rows each).
Feature rows are routed all-to-all by label bucket (host computes the
bucket/sort metadata; each core receives the feature rows whose labels land
in its bucket, in original row order).  On device, each 128-center tile
gathers its feature rows via indirect DMA into a [128 rows, 257] tile
(column 256 preset to 1.0 to produce counts), multiplies with a one-hot
matrix (built on-device from iota + per-row slot ids; value 0.1) on the
tensor engine to produce per-center 0.1*featsum and 0.1*count in PSUM, then
combines with the centers tile and writes the output shard contiguously.
"""
import sys
import types
import numpy as np

if '/opt/trn_rl_repo' not in sys.path:
    sys.path.insert(0, '/opt/trn_rl_repo')

import concourse.bass as bass
import concourse.mybir as mybir
import concourse.tile as tile
from concourse import bass_utils
from concourse import library_config

ALPHA = 0.9
SCALE = 1.0 - ALPHA  # 0.1
IOTA_MAT = np.tile(np.arange(128, dtype=np.float32), (128, 1))
N_CORES = 8
B, D, N = 65536, 256, 100000
NS = N // N_CORES  # centers per core
P = 128

F32 = mybir.dt.float32
I32 = mybir.dt.int32
I16 = mybir.dt.int16


def _patch_drain_and_barrier():
    """This walrus build encodes at most one sync-wait on the CTRL-format
    Drain instruction; split the Tile exit drain's waits across single-wait
    sync nops."""
    if getattr(tile.TileContext, '_drain_patched', False):
        return

    def _drain_and_barrier(self, tick_clock, wait_clock):
        from concourse.tile import ScopedClock
        nc = self.nc
        drain_inst = nc.sync.drain()
        wait_clock.add_sem_waits(
            drain_inst.ins, ScopedClock({None: tick_clock.global_clock})
        )
        si = drain_inst.ins.sync_info
        waits = list(si.on_wait) if si and si.on_wait else []
        if len(waits) > 1:
            si.on_wait.clear()
            si.on_wait.append(waits[0])
            for w in waits[1:]:
                nop = nc.sync.nop()
                nsi = nop.ins.sync_info
                if nsi is None:
                    nop.ins.sync_info = mybir.SyncInfo(on_wait=[w], on_update=[])
                else:
                    nsi.on_wait.append(w)
        nc.all_engine_barrier()
        popped = nc._tile_sem_poison_stack.pop()
        assert popped is self._sem_poison
        nc.clear_and_free_semaphores(list(self.sems.allocated().values()))
        nc.all_engine_barrier()

    tile.TileContext._drain_and_barrier = _drain_and_barrier
    tile.TileContext._drain_patched = True


_patch_drain_and_barrier()


def _split_multi_waits(nc):
    """This walrus build encodes only ONE sync-wait per instruction (any
    format).  Hoist every extra wait onto an InstNoOp inserted immediately
    before the instruction on the same engine (per-engine program order
    within a block makes the nops' waits complete first)."""
    for f in nc.m.functions:
        for bb in f.blocks:
            new_insts = []
            for inst in bb.instructions:
                si = inst.sync_info
                waits = list(si.on_wait) if si and si.on_wait else []
                if len(waits) > 1:
                    si.on_wait.clear()
                    for w in waits[:-1]:
                        nop = mybir.InstNoOp(
                            name=nc.get_next_instruction_name(), ins=[], outs=[]
                        )
                        nop.engine = inst.engine
                        nop.sync_info = mybir.SyncInfo(on_wait=[w], on_update=[])
                        nc.register_instruction(nop, overwrite=True)
                        new_insts.append(nop)
                    si.on_wait.append(waits[-1])
                new_insts.append(inst)
            bb.instructions[:] = new_insts


def build_routing(labels, n_cores=N_CORES, ns=NS, p=P, cap_cols=8):
    """Host-side sharding metadata with packed gather columns.

    Tiles of 128 centers are laid back-to-back in the gather position
    space at m_t = max-over-cores row-count granularity (so the layout is
    identical across cores), then cut into 128-position columns grouped
    into chunks of at most cap_cols columns.  A tile spanning multiple
    columns contributes one (tile, column) matmul incidence per column.

    Returns (shard_rows, gidx_all, slots_all, chunks) where
      chunks: list of (ncols, [(t, n_inc_cols, start_off), ...]) with
        start_off = tile's first position offset within the chunk.
      gidx_all[k]: int16 wrapped gather indices [128, POS/16]
      slots_all[k]: f32 [128, n_incidences_total]
    """
    labels = np.asarray(labels).astype(np.int64).ravel()
    t_tiles = (ns + p - 1) // p
    cap = cap_cols * p
    shard_rows, loc_sorted, lidx_sorted = [], [], []
    for k in range(n_cores):
        lo = k * ns
        rows = np.nonzero((labels >= lo) & (labels < lo + ns))[0]
        loc = labels[rows] - lo
        order = np.argsort(loc, kind='stable')
        shard_rows.append(rows)
        loc_sorted.append(loc[order])
        lidx_sorted.append(order.astype(np.int64))

    r = np.zeros((n_cores, t_tiles), dtype=np.int64)
    for k in range(n_cores):
        tl = loc_sorted[k] // p
        cnt = np.bincount(tl, minlength=t_tiles)
        r[k] = cnt[:t_tiles]
    m = np.maximum(1, r.max(axis=0))  # positions per tile, shared

    # chunk layout (shared across cores)
    chunks = []       # (ncols, [(t, c0, c1, start_off)])
    cur, fill = [], 0
    for t in range(t_tiles):
        mt = int(m[t])
        if fill + mt > cap and cur:
            chunks.append((-(-fill // p), cur))
            cur, fill = [], 0
        c0, c1 = fill // p, (fill + mt - 1) // p
        cur.append((t, c0, c1, fill))
        fill += mt
    if cur:
        chunks.append((-(-fill // p), cur))

    pos_total = sum(nc_ * p for nc_, _ in chunks)
    n_inc = sum(c1 - c0 + 1 for _, tl in chunks for (_, c0, c1, _) in tl)

    gidx_all, slots_all = [], []
    for k in range(n_cores):
        starts = np.searchsorted(loc_sorted[k] // p, np.arange(t_tiles))
        gflat = np.zeros(pos_total, dtype=np.int64)
        slots = np.full((p, n_inc), -1.0, dtype=np.float32)
        inc = 0
        chunk_base = 0
        for ncols, tl in chunks:
            for (t, c0, c1, off) in tl:
                mt = int(m[t]); rk = int(r[k, t]); s0 = int(starts[t])
                lidx = lidx_sorted[k][s0:s0 + rk]
                slot = (loc_sorted[k][s0:s0 + rk] - t * p).astype(np.float32)
                # fill gather positions for the real rows of this tile
                gflat[chunk_base + off: chunk_base + off + rk] = lidx
                for c in range(c0, c1 + 1):
                    # tile-local indices i covered by column c
                    i_lo = max(0, c * p - off)
                    i_hi = min(mt, (c + 1) * p - off)
                    pr = np.arange(i_lo, min(i_hi, rk))
                    if len(pr):
                        slots[off - c * p + pr, inc] = slot[pr]
                    inc += 1
            chunk_base += ncols * p
        assert inc == n_inc
        assert gflat.max(initial=0) < 32768
        wrapped = gflat.reshape(pos_total // 16, 16).T.astype(np.int16)
        gidx_all.append(np.tile(wrapped, (8, 1)))
        slots_all.append(slots)
    return shard_rows, gidx_all, slots_all, chunks


def build_program(chunks, n_inc, pos_total, fpad, ns=NS, d=D,
                  swdge_queues=2, single_packet=True):
    """Build the (SPMD-shared) Bass program for a packed chunk layout."""
    p = P
    fw = d + 64  # feature-shard row width: 256 features + 0.1-col + pad
    nc = bass.Bass(num_swdge_queues=swdge_queues)
    feats = nc.declare_dram_parameter('feats', [fpad, fw], F32, isOutput=False)
    centers = nc.declare_dram_parameter('centers', [ns, d], F32, isOutput=False)
    gidx_d = nc.declare_dram_parameter('gidx', [p, pos_total // 16], I16, isOutput=False)
    slots_d = nc.declare_dram_parameter('slots', [p, n_inc], F32, isOutput=False)
    iotam_d = nc.declare_dram_parameter('iotam', [p, p], F32, isOutput=False)
    out = nc.declare_dram_parameter('out', [ns, d], F32, isOutput=True)

    W = d + 1  # psum width: 256 featsum cols + 1 count col

    with tile.TileContext(nc) as tc:
        with (
            tc.tile_pool(name='const', bufs=1) as cpool,
            tc.tile_pool(name='gather', bufs=8) as gpool,
            tc.tile_pool(name='cent', bufs=2) as centpool,
            tc.tile_pool(name='outp', bufs=2) as opool,
            tc.tile_pool(name='oh', bufs=8) as ohpool,
            tc.tile_pool(name='scale', bufs=6) as spool,
            tc.tile_pool(name='psum', bufs=8, space='PSUM') as pspool,
        ):
            nc.gpsimd.load_library(library_config.mlp)
            iota_f = cpool.tile([p, p], F32)
            nc.sync.dma_start(out=iota_f[:], in_=iotam_d[:])
            gidx_sb = cpool.tile([p, pos_total // 16], I16)
            slots_sb = cpool.tile([p, n_inc], F32)
            nc.sync.dma_start(out=gidx_sb[:], in_=gidx_d[:])
            nc.sync.dma_start(out=slots_sb[:], in_=slots_d[:])

            inc = 0
            col0 = 0
            for ci, (ncols, tlist) in enumerate(chunks):
                nidx = ncols * p
                t_first, t_last = tlist[0][0], tlist[-1][0]
                nct_chunk = t_last - t_first + 1
                rows0 = t_first * p
                crows = min(ns, (t_last + 1) * p) - rows0
                full = (crows == nct_chunk * p)

                gbuf = gpool.tile([p, ncols * fw], F32, tag='gbuf')
                g3 = gbuf[:].rearrange('p (c w) -> p c w', w=fw)
                nc.gpsimd.dma_gather(
                    out_ap=g3[:, :, :],
                    in_ap=feats[:],
                    idxs_ap=gidx_sb[:, col0 * 8:(col0 + ncols) * 8],
                    num_idxs=nidx,
                    num_idxs_reg=nidx,
                    elem_size=fw,
                    queue_num=ci % swdge_queues,
                    single_packet=single_packet,
                )
                cload = centpool.tile([p, nct_chunk * d], F32, tag='cent')
                ostage = opool.tile([p, nct_chunk * d], F32, tag='ostage')
                if full:
                    nc.sync.dma_start(
                        out=cload[:].rearrange('p (t w) -> p t w', w=d),
                        in_=centers[rows0:rows0 + crows, :].rearrange(
                            '(t p) w -> p t w', p=p),
                    )
                for (t, c0, c1, off) in tlist:
                    tloc = t - t_first
                    pt = min(p, ns - t * p)
                    if not full:
                        nc.sync.dma_start(
                            out=cload[:pt, tloc * d:(tloc + 1) * d],
                            in_=centers[t * p:t * p + pt, :])
                    ps = pspool.tile([p, W], F32, tag='ps')
                    for c in range(c0, c1 + 1):
                        oh = ohpool.tile([p, p], F32, tag='oh')
                        nc.vector.tensor_tensor(
                            oh[:], iota_f[:],
                            slots_sb[:, inc:inc + 1].to_broadcast([p, p]),
                            op=mybir.AluOpType.is_equal,
                        )
                        nc.tensor.matmul(
                            ps[:], lhsT=oh[:],
                            rhs=gbuf[:, c * fw:c * fw + W],
                            start=(c == c0), stop=(c == c1),
                        )
                        inc += 1
                    # scale_vec = 1 - 0.1*count  (psum col d holds 0.1*count)
                    scale = spool.tile([p, 1], F32, tag='scale')
                    nc.scalar.activation(
                        scale[:], ps[:, d:],
                        mybir.ActivationFunctionType.Identity,
                        bias=1.0, scale=-1.0,
                    )
                    # out = centers * scale_vec  (ACT)  + 0.1*featsum  (DVE)
                    osl = ostage[:pt, tloc * d:(tloc + 1) * d]
                    nc.scalar.activation(
                        osl, cload[:pt, tloc * d:(tloc + 1) * d],
                        mybir.ActivationFunctionType.Identity,
                        bias=0.0, scale=scale[:pt, :],
                    )
                    nc.vector.tensor_tensor(
                        osl, osl, ps[:pt, 0:d], op=mybir.AluOpType.add,
                    )
                    if not full:
                        nc.scalar.dma_start(
                            out=out[t * p:t * p + pt, :],
                            in_=ostage[:pt, tloc * d:(tloc + 1) * d])
                if full:
                    nc.scalar.dma_start(
                        out=out[rows0:rows0 + crows, :].rearrange(
                            '(t p) w -> p t w', p=p),
                        in_=ostage[:].rearrange('p (t w) -> p t w', w=d),
                    )
                col0 += ncols
    _split_multi_waits(nc)
    # encode .instr bytes for extended-ISA instructions (dma_gather,
    # library reload) — bacc normally does this; raw Bass+Tile must not skip
    # it or walrus fails with "ISA wrong length"
    mybir.codegen_inst_isa_subclasses(nc)
    return nc


_PROGRAM_CACHE = {}

# test-harness knobs: when TRACE is set, pass trace=True through to
# run_bass_kernel_spmd and stash the BassKernelResults in LAST_RESULTS.
TRACE = False
TRACE_TMPDIR = None
LAST_RESULTS = None


def _get_program(chunks_key, n_inc, pos_total, fpad):
    key = (chunks_key, n_inc, pos_total, fpad)
    if key not in _PROGRAM_CACHE:
        chunks = [(ncols, list(tl)) for ncols, tl in chunks_key]
        _PROGRAM_CACHE[key] = build_program(chunks, n_inc, pos_total, fpad)
    return _PROGRAM_CACHE[key]


def kernel(features, labels, centers):
    features = np.ascontiguousarray(np.asarray(features), dtype=np.float32)
    centers_np = np.ascontiguousarray(np.asarray(centers), dtype=np.float32)
    labels_np = np.asarray(labels)

    shard_rows, gidx_all, slots_all, chunks = build_routing(labels_np)
    n_inc = slots_all[0].shape[1]
    pos_total = gidx_all[0].shape[1] * 16
    fpad = max(1, max(len(r) for r in shard_rows))

    chunks_key = tuple(
        (ncols, tuple(tl)) for ncols, tl in chunks
    )
    nc = _get_program(chunks_key, n_inc, pos_total, fpad)

    in_maps = []
    for k in range(N_CORES):
        # 0.1-scaled shard (folds the (1-alpha) factor into data prep) with a
        # 0.1-valued ones column at D for on-device counts
        fshard = np.zeros((fpad, D + 64), dtype=np.float32)
        rows = shard_rows[k]
        fshard[: len(rows), :D] = SCALE * features[rows]
        fshard[:, D] = SCALE
        in_maps.append({
            'feats': fshard,
            'centers': centers_np[k * NS:(k + 1) * NS],
            'gidx': gidx_all[k],
            'slots': slots_all[k],
            'iotam': IOTA_MAT,
        })

    kwargs = {}
    if TRACE:
        kwargs['trace'] = True
        if TRACE_TMPDIR:
            kwargs['tmpdir'] = TRACE_TMPDIR
    res = bass_utils.run_bass_kernel_spmd(
        nc, in_maps, core_ids=list(range(N_CORES)), **kwargs
    )
    global LAST_RESULTS
    LAST_RESULTS = res
    out = np.concatenate([res.results[k]['out'] for k in range(N_CORES)], axis=0)
    return out


# revision 25
# speedup vs baseline: 1.2187x; 1.2187x over previous
"""Center-update (scatter-add) kernel for Trainium2, 8 NeuronCores.

Math: given features [B, D], labels [B], centers [N, D]:
    diff        = (ALPHA - 1) * (centers[labels] - features)
    new_centers = centers.at[labels].add(diff)
which reduces per center row n to
    new_centers[n] = centers[n] * (1 - 0.1*count[n]) + 0.1 * featsum[n]
with count = histogram(labels), featsum = segment-sum of features by label.

Sharding: centers are sharded along N across the 8 cores (12500 rows each).
Feature rows are routed all-to-all by label bucket (host computes the
bucket/sort metadata; each core receives the feature rows whose labels land
in its bucket, in original row order).  On device, each 128-center tile
gathers its feature rows via indirect DMA into a [128 rows, 257] tile
(column 256 preset to 1.0 to produce counts), multiplies with a one-hot
matrix (built on-device from iota + per-row slot ids; value 0.1) on the
tensor engine to produce per-center 0.1*featsum and 0.1*count in PSUM, then
combines with the centers tile and writes the output shard contiguously.
"""
import sys
import types
import numpy as np

if '/opt/trn_rl_repo' not in sys.path:
    sys.path.insert(0, '/opt/trn_rl_repo')

import concourse.bass as bass
import concourse.mybir as mybir
import concourse.tile as tile
from concourse import bass_utils
from concourse import library_config

ALPHA = 0.9
SCALE = 1.0 - ALPHA  # 0.1
IOTA_MAT = np.tile(np.arange(128, dtype=np.float32), (128, 1))
N_CORES = 8
B, D, N = 65536, 256, 100000
NS = N // N_CORES  # centers per core
P = 128

F32 = mybir.dt.float32
I32 = mybir.dt.int32
I16 = mybir.dt.int16


def _patch_drain_and_barrier():
    """This walrus build encodes at most one sync-wait on the CTRL-format
    Drain instruction; split the Tile exit drain's waits across single-wait
    sync nops."""
    if getattr(tile.TileContext, '_drain_patched', False):
        return

    def _drain_and_barrier(self, tick_clock, wait_clock):
        from concourse.tile import ScopedClock
        nc = self.nc
        drain_inst = nc.sync.drain()
        wait_clock.add_sem_waits(
            drain_inst.ins, ScopedClock({None: tick_clock.global_clock})
        )
        si = drain_inst.ins.sync_info
        waits = list(si.on_wait) if si and si.on_wait else []
        if len(waits) > 1:
            si.on_wait.clear()
            si.on_wait.append(waits[0])
            for w in waits[1:]:
                nop = nc.sync.nop()
                nsi = nop.ins.sync_info
                if nsi is None:
                    nop.ins.sync_info = mybir.SyncInfo(on_wait=[w], on_update=[])
                else:
                    nsi.on_wait.append(w)
        nc.all_engine_barrier()
        popped = nc._tile_sem_poison_stack.pop()
        assert popped is self._sem_poison
        nc.clear_and_free_semaphores(list(self.sems.allocated().values()))
        nc.all_engine_barrier()

    tile.TileContext._drain_and_barrier = _drain_and_barrier
    tile.TileContext._drain_patched = True


_patch_drain_and_barrier()


def _split_multi_waits(nc):
    """This walrus build encodes only ONE sync-wait per instruction (any
    format).  Hoist every extra wait onto an InstNoOp inserted immediately
    before the instruction on the same engine (per-engine program order
    within a block makes the nops' waits complete first)."""
    for f in nc.m.functions:
        for bb in f.blocks:
            new_insts = []
            for inst in bb.instructions:
                si = inst.sync_info
                waits = list(si.on_wait) if si and si.on_wait else []
                if len(waits) > 1:
                    si.on_wait.clear()
                    for w in waits[:-1]:
                        nop = mybir.InstNoOp(
                            name=nc.get_next_instruction_name(), ins=[], outs=[]
                        )
                        nop.engine = inst.engine
                        nop.sync_info = mybir.SyncInfo(on_wait=[w], on_update=[])
                        nc.register_instruction(nop, overwrite=True)
                        new_insts.append(nop)
                    si.on_wait.append(waits[-1])
                new_insts.append(inst)
            bb.instructions[:] = new_insts


def build_routing(labels, n_cores=N_CORES, ns=NS, p=P, cap_cols=8):
    """Host-side sharding metadata with packed gather columns.

    Tiles of 128 centers are laid back-to-back in the gather position
    space at m_t = max-over-cores row-count granularity (so the layout is
    identical across cores), then cut into 128-position columns grouped
    into chunks of at most cap_cols columns.  A tile spanning multiple
    columns contributes one (tile, column) matmul incidence per column.

    Returns (shard_rows, gidx_all, slots_all, chunks) where
      chunks: list of (ncols, [(t, n_inc_cols, start_off), ...]) with
        start_off = tile's first position offset within the chunk.
      gidx_all[k]: int16 wrapped gather indices [128, POS/16]
      slots_all[k]: f32 [128, n_incidences_total]
    """
    labels = np.asarray(labels).astype(np.int64).ravel()
    t_tiles = (ns + p - 1) // p
    cap = cap_cols * p
    shard_rows, loc_sorted, lidx_sorted = [], [], []
    for k in range(n_cores):
        lo = k * ns
        rows = np.nonzero((labels >= lo) & (labels < lo + ns))[0]
        loc = labels[rows] - lo
        order = np.argsort(loc, kind='stable')
        shard_rows.append(rows)
        loc_sorted.append(loc[order])
        lidx_sorted.append(order.astype(np.int64))

    r = np.zeros((n_cores, t_tiles), dtype=np.int64)
    for k in range(n_cores):
        tl = loc_sorted[k] // p
        cnt = np.bincount(tl, minlength=t_tiles)
        r[k] = cnt[:t_tiles]
    m = np.maximum(1, r.max(axis=0))  # positions per tile, shared

    # chunk layout (shared across cores)
    chunks = []       # (ncols, [(t, c0, c1, start_off)])
    cur, fill = [], 0
    for t in range(t_tiles):
        mt = int(m[t])
        if fill + mt > cap and cur:
            chunks.append((-(-fill // p), cur))
            cur, fill = [], 0
        c0, c1 = fill // p, (fill + mt - 1) // p
        cur.append((t, c0, c1, fill))
        fill += mt
    if cur:
        chunks.append((-(-fill // p), cur))

    pos_total = sum(nc_ * p for nc_, _ in chunks)
    n_inc = sum(c1 - c0 + 1 for _, tl in chunks for (_, c0, c1, _) in tl)

    gidx_all, slots_all = [], []
    for k in range(n_cores):
        starts = np.searchsorted(loc_sorted[k] // p, np.arange(t_tiles))
        gflat = np.zeros(pos_total, dtype=np.int64)
        slots = np.full((p, n_inc), -1.0, dtype=np.float32)
        inc = 0
        chunk_base = 0
        for ncols, tl in chunks:
            for (t, c0, c1, off) in tl:
                mt = int(m[t]); rk = int(r[k, t]); s0 = int(starts[t])
                lidx = lidx_sorted[k][s0:s0 + rk]
                slot = (loc_sorted[k][s0:s0 + rk] - t * p).astype(np.float32)
                # fill gather positions for the real rows of this tile
                gflat[chunk_base + off: chunk_base + off + rk] = lidx
                for c in range(c0, c1 + 1):
                    # tile-local indices i covered by column c
                    i_lo = max(0, c * p - off)
                    i_hi = min(mt, (c + 1) * p - off)
                    pr = np.arange(i_lo, min(i_hi, rk))
                    if len(pr):
                        slots[off - c * p + pr, inc] = slot[pr]
                    inc += 1
            chunk_base += ncols * p
        assert inc == n_inc
        assert gflat.max(initial=0) < 32768
        wrapped = gflat.reshape(pos_total // 16, 16).T.astype(np.int16)
        gidx_all.append(np.tile(wrapped, (8, 1)))
        slots_all.append(slots)
    return shard_rows, gidx_all, slots_all, chunks


def build_program(chunks, n_inc, pos_total, fpad, ns=NS, d=D,
                  swdge_queues=2, single_packet=True):
    """Build the (SPMD-shared) Bass program for a packed chunk layout."""
    p = P
    fw = d + 64  # feature-shard row width: 256 features + 0.1-col + pad
    nc = bass.Bass(num_swdge_queues=swdge_queues)
    feats = nc.declare_dram_parameter('feats', [fpad, fw], F32, isOutput=False)
    centers = nc.declare_dram_parameter('centers', [ns, d], F32, isOutput=False)
    gidx_d = nc.declare_dram_parameter('gidx', [p, pos_total // 16], I16, isOutput=False)
    slots_d = nc.declare_dram_parameter('slots', [p, n_inc], F32, isOutput=False)
    iotam_d = nc.declare_dram_parameter('iotam', [p, p], F32, isOutput=False)
    out = nc.declare_dram_parameter('out', [ns, d], F32, isOutput=True)

    W = d + 1  # psum width: 256 featsum cols + 1 count col

    with tile.TileContext(nc) as tc:
        with (
            tc.tile_pool(name='const', bufs=1) as cpool,
            tc.tile_pool(name='gather', bufs=6) as gpool,
            tc.tile_pool(name='cent', bufs=3) as centpool,
            tc.tile_pool(name='outp', bufs=3) as opool,
            tc.tile_pool(name='oh', bufs=8) as ohpool,
            tc.tile_pool(name='scale', bufs=6) as spool,
            tc.tile_pool(name='psum', bufs=8, space='PSUM') as pspool,
        ):
            nc.gpsimd.load_library(library_config.mlp)
            iota_f = cpool.tile([p, p], F32)
            nc.sync.dma_start(out=iota_f[:], in_=iotam_d[:])
            gidx_sb = cpool.tile([p, pos_total // 16], I16)
            slots_sb = cpool.tile([p, n_inc], F32)
            nc.sync.dma_start(out=gidx_sb[:], in_=gidx_d[:])
            nc.sync.dma_start(out=slots_sb[:], in_=slots_d[:])

            inc = 0
            col0 = 0
            for ci, (ncols, tlist) in enumerate(chunks):
                nidx = ncols * p
                t_first, t_last = tlist[0][0], tlist[-1][0]
                nct_chunk = t_last - t_first + 1
                rows0 = t_first * p
                crows = min(ns, (t_last + 1) * p) - rows0
                full = (crows == nct_chunk * p)

                gbuf = gpool.tile([p, ncols * fw], F32, tag='gbuf')
                g3 = gbuf[:].rearrange('p (c w) -> p c w', w=fw)
                nc.gpsimd.dma_gather(
                    out_ap=g3[:, :, :],
                    in_ap=feats[:],
                    idxs_ap=gidx_sb[:, col0 * 8:(col0 + ncols) * 8],
                    num_idxs=nidx,
                    num_idxs_reg=nidx,
                    elem_size=fw,
                    queue_num=ci % swdge_queues,
                    single_packet=single_packet,
                )
                cload = centpool.tile([p, nct_chunk * d], F32, tag='cent')
                ostage = opool.tile([p, nct_chunk * d], F32, tag='ostage')
                if full:
                    nc.sync.dma_start(
                        out=cload[:].rearrange('p (t w) -> p t w', w=d),
                        in_=centers[rows0:rows0 + crows, :].rearrange(
                            '(t p) w -> p t w', p=p),
                    )
                for (t, c0, c1, off) in tlist:
                    tloc = t - t_first
                    pt = min(p, ns - t * p)
                    if not full:
                        nc.sync.dma_start(
                            out=cload[:pt, tloc * d:(tloc + 1) * d],
                            in_=centers[t * p:t * p + pt, :])
                    ps = pspool.tile([p, W], F32, tag='ps')
                    for c in range(c0, c1 + 1):
                        oh = ohpool.tile([p, p], F32, tag='oh')
                        nc.vector.tensor_tensor(
                            oh[:], iota_f[:],
                            slots_sb[:, inc:inc + 1].to_broadcast([p, p]),
                            op=mybir.AluOpType.is_equal,
                        )
                        nc.tensor.matmul(
                            ps[:], lhsT=oh[:],
                            rhs=gbuf[:, c * fw:c * fw + W],
                            start=(c == c0), stop=(c == c1),
                        )
                        inc += 1
                    # scale_vec = 1 - 0.1*count  (psum col d holds 0.1*count)
                    scale = spool.tile([p, 1], F32, tag='scale')
                    nc.scalar.activation(
                        scale[:], ps[:, d:],
                        mybir.ActivationFunctionType.Identity,
                        bias=1.0, scale=-1.0,
                    )
                    # out = centers * scale_vec  (ACT)  + 0.1*featsum  (DVE)
                    osl = ostage[:pt, tloc * d:(tloc + 1) * d]
                    nc.scalar.activation(
                        osl, cload[:pt, tloc * d:(tloc + 1) * d],
                        mybir.ActivationFunctionType.Identity,
                        bias=0.0, scale=scale[:pt, :],
                    )
                    nc.vector.tensor_tensor(
                        osl, osl, ps[:pt, 0:d], op=mybir.AluOpType.add,
                    )
                    if not full:
                        nc.scalar.dma_start(
                            out=out[t * p:t * p + pt, :],
                            in_=ostage[:pt, tloc * d:(tloc + 1) * d])
                if full:
                    nc.scalar.dma_start(
                        out=out[rows0:rows0 + crows, :].rearrange(
                            '(t p) w -> p t w', p=p),
                        in_=ostage[:].rearrange('p (t w) -> p t w', w=d),
                    )
                col0 += ncols
    _split_multi_waits(nc)
    # encode .instr bytes for extended-ISA instructions (dma_gather,
    # library reload) — bacc normally does this; raw Bass+Tile must not skip
    # it or walrus fails with "ISA wrong length"
    mybir.codegen_inst_isa_subclasses(nc)
    return nc


_PROGRAM_CACHE = {}

# test-harness knobs: when TRACE is set, pass trace=True through to
# run_bass_kernel_spmd and stash the BassKernelResults in LAST_RESULTS.
TRACE = False
TRACE_TMPDIR = None
LAST_RESULTS = None


def _get_program(chunks_key, n_inc, pos_total, fpad):
    key = (chunks_key, n_inc, pos_total, fpad)
    if key not in _PROGRAM_CACHE:
        chunks = [(ncols, list(tl)) for ncols, tl in chunks_key]
        _PROGRAM_CACHE[key] = build_program(chunks, n_inc, pos_total, fpad)
    return _PROGRAM_CACHE[key]


def kernel(features, labels, centers):
    features = np.ascontiguousarray(np.asarray(features), dtype=np.float32)
    centers_np = np.ascontiguousarray(np.asarray(centers), dtype=np.float32)
    labels_np = np.asarray(labels)

    shard_rows, gidx_all, slots_all, chunks = build_routing(labels_np)
    n_inc = slots_all[0].shape[1]
    pos_total = gidx_all[0].shape[1] * 16
    fpad = max(1, max(len(r) for r in shard_rows))

    chunks_key = tuple(
        (ncols, tuple(tl)) for ncols, tl in chunks
    )
    nc = _get_program(chunks_key, n_inc, pos_total, fpad)

    in_maps = []
    for k in range(N_CORES):
        # 0.1-scaled shard (folds the (1-alpha) factor into data prep) with a
        # 0.1-valued ones column at D for on-device counts
        fshard = np.zeros((fpad, D + 64), dtype=np.float32)
        rows = shard_rows[k]
        fshard[: len(rows), :D] = SCALE * features[rows]
        fshard[:, D] = SCALE
        in_maps.append({
            'feats': fshard,
            'centers': centers_np[k * NS:(k + 1) * NS],
            'gidx': gidx_all[k],
            'slots': slots_all[k],
            'iotam': IOTA_MAT,
        })

    kwargs = {}
    if TRACE:
        kwargs['trace'] = True
        if TRACE_TMPDIR:
            kwargs['tmpdir'] = TRACE_TMPDIR
    res = bass_utils.run_bass_kernel_spmd(
        nc, in_maps, core_ids=list(range(N_CORES)), **kwargs
    )
    global LAST_RESULTS
    LAST_RESULTS = res
    out = np.concatenate([res.results[k]['out'] for k in range(N_CORES)], axis=0)
    return out
